# revision 3
# baseline (speedup 1.0000x reference)
"""Trainium2 Bass kernel for nn_CrossAttention (dual-stream cross attention).

Reference (per batch element b, D=768, H=12, Dh=64, N=1024):
  m1 = LN(x1) ; m2 = LN(x2)
  q1,k1,v1 = m1 @ {wq1,wk1,wv1} + b ; q2,k2,v2 = m2 @ {wq2,wk2,wv2} + b
  out1 = softmax(q1 k2^T / 8) v2 @ wo1 + bo1
  out2 = softmax(q2 k1^T / 8) v1 @ wo2 + bo2

Sharding: pure data-parallel over batch B=8 -> 8 NeuronCores, no collectives.

Host prep per core: x transposed to [D, N] fp32; weights cast to bf16
(wq/wk/wv as [D,D] bf16; wo in head-major [64, 12, D] bf16).

On-chip layout (per core):
  xT fp32 [128, 6, 1024]  (d on partitions)
  LN stats via ones-matmul chains over bf16 x and x^2 (PE reduces over d);
  row stats at partition 0, broadcast via gpsimd.partition_broadcast.
  mT bf16 [128, 6, 1024]
  q/k projections per e-chunk (T-layout, psum accumulate over 6 d-chunks)
  v projection in N-layout into v_aug [128, 8, 12*65] with a ones column
  per head ([v | 1]) so PV yields both o^T (rows 0..63) and the softmax
  denominator (row 64) in one accumulation chain.
  scores S^T[m,n] per head via K=64 matmuls (head pair at partition
  offsets 0/64 -> concurrent row-groups); exp on ScalarE from 2-bank psum.
  o^T normalized by broadcast reciprocal denominator, stored bf16,
  bounced via DRAM, reloaded as [64, 12, 128] slices for the output
  projection (12 accumulating K=64 matmuls), + bias term
  c = bo + bv' @ wo computed on chip.
"""

import sys

if "/opt/trn_rl_repo" not in sys.path:
    sys.path.insert(0, "/opt/trn_rl_repo")

import numpy as np
import ml_dtypes

import concourse.bass as bass
import concourse.tile as tile
from concourse import bacc, mybir
from concourse.bass_utils import run_bass_kernel_spmd

P = 128
D = 768
N = 1024
H = 12
DH = 64
DC = D // P  # 6 d-chunks
NC = N // P  # 8 n-chunks
EPS = 1e-5
F32 = mybir.dt.float32
BF16 = mybir.dt.bfloat16


def build_nc():
    nc = bacc.Bacc("TRN2", target_bir_lowering=False, debug=False, num_devices=8)

    # ---- DRAM I/O ----
    x1T = nc.dram_tensor("x1T", [D, N], F32, kind="ExternalInput").ap()
    x2T = nc.dram_tensor("x2T", [D, N], F32, kind="ExternalInput").ap()
    wq = {}
    for s in (1, 2):
        for p_ in ("q", "k", "v"):
            wq[(p_, s)] = nc.dram_tensor(f"w{p_}{s}", [D, D], BF16, kind="ExternalInput").ap()
    wo_hm = {
        s: nc.dram_tensor(f"wo{s}hm", [DH, H, D], BF16, kind="ExternalInput").ap()
        for s in (1, 2)
    }
    bias = {}
    for s in (1, 2):
        for p_ in ("q", "k"):
            bias[(p_, s)] = nc.dram_tensor(f"b{p_}{s}", [D], F32, kind="ExternalInput").ap()
    bv_hm = {s: nc.dram_tensor(f"bv{s}hm", [DH, H], BF16, kind="ExternalInput").ap() for s in (1, 2)}
    bo = {s: nc.dram_tensor(f"bo{s}", [D], F32, kind="ExternalInput").ap() for s in (1, 2)}
    lng = {s: nc.dram_tensor(f"ln{s}_g", [D], F32, kind="ExternalInput").ap() for s in (1, 2)}
    lnb = {s: nc.dram_tensor(f"ln{s}_b", [D], F32, kind="ExternalInput").ap() for s in (1, 2)}
    out = {s: nc.dram_tensor(f"out{s}", [N, D], F32, kind="ExternalOutput").ap() for s in (1, 2)}

    # DRAM scratch for o^T bounce: [12, 64, 1024] bf16 per stream
    oTd = {s: nc.dram_tensor(f"oTd{s}", [H, DH, N], BF16).ap() for s in (1, 2)}

    xT = {1: x1T, 2: x2T}

    with tile.TileContext(nc) as tc:
        with (
            tc.tile_pool(name="cst", bufs=1) as cst,
            tc.tile_pool(name="big", bufs=2) as big,
            tc.tile_pool(name="work", bufs=2) as work,
            tc.tile_pool(name="attn", bufs=2) as attn,
            tc.tile_pool(name="ps", bufs=2, space="PSUM") as ps,
            tc.tile_pool(name="pso", bufs=4, space="PSUM") as pso,
        ):
            ones_b = cst.tile([P, 1], BF16)
            nc.vector.memset(ones_b, 1.0)
            eps_t = cst.tile([P, 1], F32)
            nc.vector.memset(eps_t, EPS)

            # per-partition param tiles [128, 6]
            lngt = {s: cst.tile([P, DC], F32, name=f"lngt{s}") for s in (1, 2)}
            lnbt = {s: cst.tile([P, DC], F32, name=f"lnbt{s}") for s in (1, 2)}
            bqt = {s: cst.tile([P, DC], F32, name=f"bqt{s}") for s in (1, 2)}
            bkt = {s: cst.tile([P, DC], F32, name=f"bkt{s}") for s in (1, 2)}
            for s in (1, 2):
                nc.sync.dma_start(lngt[s], lng[s].rearrange("(c p) -> p c", p=P))
                nc.sync.dma_start(lnbt[s], lnb[s].rearrange("(c p) -> p c", p=P))
                nc.sync.dma_start(bqt[s], bias[("q", s)].rearrange("(c p) -> p c", p=P))
                nc.sync.dma_start(bkt[s], bias[("k", s)].rearrange("(c p) -> p c", p=P))

            # ---------- Phase A: LayerNorm -> mT (bf16, T-layout) ----------
            mT = {s: cst.tile([P, DC, N], BF16, name=f"mT{s}") for s in (1, 2)}
            for s in (1, 2):
                xs = big.tile([P, DC, N], F32, tag="slab24", bufs=1)
                nc.sync.dma_start(xs, xT[s].rearrange("(c p) n -> p c n", p=P))

                psum_sum = pso.tile([P, 512], F32, tag="p512", name="psum_sum0")
                psum_sum1 = pso.tile([P, 512], F32, tag="p512", name="psum_sum1")
                psum_sq = pso.tile([P, 512], F32, tag="p512", name="psum_sq0")
                psum_sq1 = pso.tile([P, 512], F32, tag="p512", name="psum_sq1")
                sums = (psum_sum, psum_sum1)
                sqs = (psum_sq, psum_sq1)
                for dc in range(DC):
                    xb = work.tile([P, N], BF16, tag="xb")
                    nc.vector.tensor_copy(xb, xs[:, dc, :])
                    xq = work.tile([P, N], BF16, tag="xq")
                    nc.scalar.activation(xq, xb, mybir.ActivationFunctionType.Square)
                    for nh in range(2):
                        nc.tensor.matmul(
                            sums[nh][0:1, :], ones_b, xb[:, nh * 512 : nh * 512 + 512],
                            start=(dc == 0), stop=(dc == DC - 1),
                        )
                        nc.tensor.matmul(
                            sqs[nh][0:1, :], ones_b, xq[:, nh * 512 : nh * 512 + 512],
                            start=(dc == 0), stop=(dc == DC - 1),
                        )

                # row stats at partition 0: rows tile [128, 3, 1024] f32
                rows = work.tile([P, 3, N], F32, tag="rows", bufs=1)
                for nh in range(2):
                    sl = slice(nh * 512, nh * 512 + 512)
                    nc.vector.tensor_scalar_mul(rows[0:1, 0, sl], sums[nh][0:1, :], 1.0 / D)
                    nc.vector.tensor_scalar_mul(rows[0:1, 1, sl], sqs[nh][0:1, :], 1.0 / D)
                # tmp = mu^2 ; var = msq - mu^2 ; sd = sqrt(var+eps) ; rinv ; mur
                nc.vector.tensor_mul(rows[0:1, 2, :], rows[0:1, 0, :], rows[0:1, 0, :])
                nc.vector.tensor_tensor(
                    rows[0:1, 1, :], rows[0:1, 1, :], rows[0:1, 2, :], mybir.AluOpType.subtract
                )
                nc.scalar.activation(
                    rows[0:1, 1, :], rows[0:1, 1, :], mybir.ActivationFunctionType.Sqrt,
                    bias=eps_t[0:1, :],
                )
                nc.vector.reciprocal(rows[0:1, 1, :], rows[0:1, 1, :])  # rinv
                nc.vector.tensor_mul(rows[0:1, 0, :], rows[0:1, 0, :], rows[0:1, 1, :])  # mur

                rb = work.tile([P, N], F32, tag="rb", bufs=1)
                nc.gpsimd.partition_broadcast(rb, rows[0:1, 1, :])
                murbf = work.tile([P, N], BF16, tag="murbf", bufs=1)
                nc.vector.tensor_copy(murbf[0:1, :], rows[0:1, 0, :])
                murb = work.tile([P, N], BF16, tag="murb", bufs=1)
                nc.gpsimd.partition_broadcast(murb, murbf[0:1, :])

                for dc in range(DC):
                    t = work.tile([P, N], BF16, tag="lnt")
                    nc.vector.tensor_mul(t, xs[:, dc, :], rb)
                    nc.vector.tensor_tensor(t, t, murb, mybir.AluOpType.subtract)
                    nc.vector.tensor_scalar(
                        mT[s][:, dc, :], t, lngt[s][:, dc : dc + 1], lnbt[s][:, dc : dc + 1],
                        mybir.AluOpType.mult, mybir.AluOpType.add,
                    )

            # ---------- Phase B1: V projections (N-layout) into v_aug ----------
            v_aug = {s: cst.tile([P, NC, H * 65], BF16, name=f"vaug{s}") for s in (1, 2)}
            for s in (1, 2):
                nc.vector.memset(
                    v_aug[s].rearrange("p m (h u) -> p m h u", u=65)[:, :, :, 64:65], 1.0
                )
                for eh, esz in ((0, 512), (1, 256)):
                    wv = work.tile([P, DC, 512], BF16, tag="wv", bufs=1)
                    nc.sync.dma_start(
                        wv[:, :, 0:esz],
                        wq[("v", s)][:, eh * 512 : eh * 512 + esz].rearrange(
                            "(c p) e -> p c e", p=P
                        ),
                    )
                    for nc_ in range(NC):
                        pv = pso.tile([P, 512], F32, tag="p512", name="pvproj")
                        for dc in range(DC):
                            nc.tensor.matmul(
                                pv[:, 0:esz],
                                mT[s][:, dc, nc_ * P : nc_ * P + P],
                                wv[:, dc, 0:esz],
                                start=(dc == 0), stop=(dc == DC - 1),
                            )
                        nh_heads = esz // DH
                        nc.vector.tensor_copy(
                            v_aug[s][:, nc_, :].rearrange("p (h u) -> p h u", u=65)[
                                :, eh * 8 : eh * 8 + nh_heads, 0:DH
                            ],
                            pv[:, 0:esz].rearrange("p (h u) -> p h u", u=DH),
                        )

            # ---------- Phases B2+C interleaved per e-chunk: q/k proj + attention ----------
            for ec in range(DC):
                # project q1,k1,q2,k2 e-chunk ec -> [128, 1024] bf16 slices
                qk_sl = {}
                for s in (1, 2):
                    for p_ in ("q", "k"):
                        wsl = work.tile([P, DC, P], BF16, tag="wqk")
                        nc.sync.dma_start(
                            wsl,
                            wq[(p_, s)][:, ec * P : ec * P + P].rearrange(
                                "(c p) e -> p c e", p=P
                            ),
                        )
                        sl = work.tile([P, N], BF16, tag=f"sl_{p_}{s}", bufs=1)
                        bt = bqt[s] if p_ == "q" else bkt[s]
                        for nh in range(2):
                            pp = pso.tile([P, 512], F32, tag="p512", name="pproj")
                            for dc in range(DC):
                                nc.tensor.matmul(
                                    pp,
                                    wsl[:, dc, :],
                                    mT[s][:, dc, nh * 512 : nh * 512 + 512],
                                    start=(dc == 0), stop=(dc == DC - 1),
                                )
                            nc.vector.tensor_scalar(
                                sl[:, nh * 512 : nh * 512 + 512], pp,
                                bt[:, ec : ec + 1], None, mybir.AluOpType.add,
                            )
                        qk_sl[(p_, s)] = sl

                # attention for head pair (2*ec, 2*ec+1) of both streams
                for s in (1, 2):
                    so = 2 if s == 1 else 1  # other stream provides k, v
                    qsl = qk_sl[("q", s)]
                    ksl = qk_sl[("k", so)]
                    # o^T psums: [h_local][nh] accumulating over mc
                    po = [
                        [pso.tile([P, 512], F32, tag="p512", name=f"po{hl}{nh}") for nh in range(2)]
                        for hl in range(2)
                    ]
                    E = {}
                    for mc in range(NC):
                        for hl in range(2):
                            b0 = hl * DH
                            pqk = ps.tile([P, N], F32, tag="s1024")
                            for nh in range(2):
                                nc.tensor.matmul(
                                    pqk[:, nh * 512 : nh * 512 + 512],
                                    ksl[b0 : b0 + DH, mc * P : mc * P + P],
                                    qsl[b0 : b0 + DH, nh * 512 : nh * 512 + 512],
                                    start=True, stop=True,
                                )
                            e = attn.tile([P, N], BF16, tag="E", bufs=4)
                            nc.scalar.activation(
                                e, pqk, mybir.ActivationFunctionType.Exp, scale=0.125
                            )
                            E[(hl, mc)] = e
                        for hl in range(2):
                            h_glob = 2 * ec + hl
                            for nh in range(2):
                                nc.tensor.matmul(
                                    po[hl][nh][0:65, :],
                                    v_aug[so][:, mc, h_glob * 65 : h_glob * 65 + 65],
                                    E[(hl, mc)][:, nh * 512 : nh * 512 + 512],
                                    start=(mc == 0), stop=(mc == NC - 1),
                                )
                    # normalize + store o^T per head
                    for hl in range(2):
                        h_glob = 2 * ec + hl
                        den = attn.tile([P, N], F32, tag="den")
                        for nh in range(2):
                            nc.vector.tensor_copy(
                                den[64:65, nh * 512 : nh * 512 + 512], po[hl][nh][64:65, :]
                            )
                        nc.sync.dma_start(den[0:1, :], den[64:65, :])
                        nc.vector.reciprocal(den[0:1, :], den[0:1, :])
                        rdb = attn.tile([P, N], F32, tag="rdb")
                        nc.gpsimd.partition_broadcast(rdb[0:DH, :], den[0:1, :])
                        oT = attn.tile([DH, N], BF16, tag="oT")
                        for nh in range(2):
                            nc.vector.tensor_mul(
                                oT[:, nh * 512 : nh * 512 + 512],
                                po[hl][nh][0:DH, :],
                                rdb[0:DH, nh * 512 : nh * 512 + 512],
                            )
                        nc.sync.dma_start(oTd[s][h_glob], oT)

            # ---------- Phase D: output projections ----------
            for s in (1, 2):
                wob = big.tile([DH, H, D], BF16, tag="slab24", bufs=1, name=f"wob{s}")
                nc.sync.dma_start(wob, wo_hm[s])
                bvb = work.tile([DH, H], BF16, tag="bvb")
                nc.sync.dma_start(bvb, bv_hm[2 if s == 1 else 1])
                bor = work.tile([P, D], F32, tag="bor", bufs=1)
                nc.sync.dma_start(bor[0:1, :], bo[s][None, :])

                # c = bo + bv' @ wo  (row at partition 0), then broadcast
                crow = work.tile([P, D], F32, tag="crow", bufs=1)
                for eh, esz in ((0, 512), (1, 256)):
                    pc = pso.tile([P, 512], F32, tag="p512", name="pc")
                    for h in range(H):
                        nc.tensor.matmul(
                            pc[0:1, 0:esz],
                            bvb[:, h : h + 1],
                            wob[:, h, eh * 512 : eh * 512 + esz],
                            start=(h == 0), stop=(h == H - 1),
                        )
                    nc.vector.tensor_add(
                        crow[0:1, eh * 512 : eh * 512 + esz],
                        pc[0:1, 0:esz],
                        bor[0:1, eh * 512 : eh * 512 + esz],
                    )
                cb = work.tile([P, D], F32, tag="cb")
                nc.gpsimd.partition_broadcast(cb, crow[0:1, :])

                for nc_ in range(NC):
                    osl = work.tile([DH, H, P], BF16, tag="osl", bufs=2)
                    nc.sync.dma_start(
                        osl, oTd[s][:, :, nc_ * P : nc_ * P + P].rearrange("h u n -> u h n")
                    )
                    outsb = work.tile([P, D], F32, tag="outsb", bufs=2)
                    for eh, esz in ((0, 512), (1, 256)):
                        pout = pso.tile([P, 512], F32, tag="p512", name="pout")
                        for h in range(H):
                            nc.tensor.matmul(
                                pout[:, 0:esz],
                                osl[:, h, :],
                                wob[:, h, eh * 512 : eh * 512 + esz],
                                start=(h == 0), stop=(h == H - 1),
                            )
                        nc.vector.tensor_add(
                            outsb[:, eh * 512 : eh * 512 + esz],
                            pout[:, 0:esz],
                            cb[:, eh * 512 : eh * 512 + esz],
                        )
                    nc.sync.dma_start(out[s][nc_ * P : nc_ * P + P, :], outsb)

    nc.compile()
    return nc


_NC_CACHE = None


def _get_nc():
    global _NC_CACHE
    if _NC_CACHE is None:
        _NC_CACHE = build_nc()
    return _NC_CACHE


def _make_in_maps(inputs):
    B = 8
    bf = ml_dtypes.bfloat16

    # host prep shared across cores (weights)
    shared = {}
    for s in (1, 2):
        for p_ in ("q", "k", "v"):
            shared[f"w{p_}{s}"] = np.ascontiguousarray(inputs[f"w{p_}{s}"]).astype(bf)
        # wo head-major: [D, D] -> [64, 12, D]
        wo = np.asarray(inputs[f"wo{s}"], np.float32)
        shared[f"wo{s}hm"] = np.ascontiguousarray(
            wo.reshape(H, DH, D).transpose(1, 0, 2)
        ).astype(bf)
        shared[f"bq{s}"] = np.asarray(inputs[f"bq{s}"], np.float32)
        shared[f"bk{s}"] = np.asarray(inputs[f"bk{s}"], np.float32)
        bv = np.asarray(inputs[f"bv{s}"], np.float32)
        shared[f"bv{s}hm"] = np.ascontiguousarray(bv.reshape(H, DH).T).astype(bf)
        shared[f"bo{s}"] = np.asarray(inputs[f"bo{s}"], np.float32)
        shared[f"ln{s}_g"] = np.asarray(inputs[f"ln{s}_g"], np.float32)
        shared[f"ln{s}_b"] = np.asarray(inputs[f"ln{s}_b"], np.float32)

    x1 = np.asarray(inputs["x1"], np.float32)
    x2 = np.asarray(inputs["x2"], np.float32)
    in_maps = []
    for b in range(B):
        m = dict(shared)
        m["x1T"] = np.ascontiguousarray(x1[b].T)
        m["x2T"] = np.ascontiguousarray(x2[b].T)
        in_maps.append(m)
    return in_maps


def kernel(**inputs):
    B = 8
    nc = _get_nc()
    in_maps = _make_in_maps(inputs)
    res = run_bass_kernel_spmd(nc, in_maps, list(range(B))).results
    out1 = np.stack([res[b]["out1"] for b in range(B)]).astype(np.float32)
    out2 = np.stack([res[b]["out2"] for b in range(B)]).astype(np.float32)
    return (out1, out2)

